# revision 5
# baseline (speedup 1.0000x reference)
"""Trainium2 Bass kernel for MCPRN (purpose-routed GRU-variant recommender).

Two SPMD launches on 8 NeuronCores, with a persistent-SBUF bridge:

Launch 1 (scan): six cores run (purpose p, batch-half h) PSRU scans as two
anti-phased B=32 sub-chains, statically software-pipelined at ~2.14us/step
(planned-phase emission; the tile scheduler follows the planned order):
  PE r-gate h-matmuls -> sigmoid(r) [Act] -> u1 = ghn*r [DVE] ->
  u2 = u1+gin [DVE] -> tanh [Act] -> an = a*n [DVE] -> next step's matmuls.
  sigmoid(i), a = cf*i, q = 1-a, h*q, h updates, and the x-side group
  matmuls all run off the critical chain in engine idle windows.
  Hidden state is kept split (h = hq + an) so the recurrent matmuls take
  hq/an as two accumulating moving operands.
  While the scan runs (latency-bound, DMA idle), the three softmax-
  pre-weighted scoring tables we_p = emb * tcw[:,p] (9.6 MB/core) stream
  into a right-side SBUF region that persists across launches.

Launch 2 (score): each core scores its ~6250-item slice reading the
pre-staged tables directly from persistent SBUF (verified same address at
build time) -- no table DMA. Six matmuls per 512-item chunk accumulate in
one PSUM bank; PSUM->SBUF copies on Act/DVE; chunked writebacks.

Host work is input staging only: emb[seq] gather, concen-softmax weights,
tcw table pre-weighting, and the final unshard/concat.
"""

import numpy as np
import ml_dtypes

import concourse.bacc as bacc
import concourse.mybir as mybir
import concourse.tile as tile
from concourse.bass import ts, ds
from concourse.bass_utils import run_bass_kernel_spmd

F32 = mybir.dt.float32
BF16 = mybir.dt.bfloat16
AF = mybir.ActivationFunctionType
OP = mybir.AluOpType

N_ITEMS = 50001
DIM = 256
TAU = 0.1
S = 50
B = 128
EPS = 0.01
BH = 64         # batch per scan core
SW = 32         # batch per sub-chain
SB = S * BH     # 3200 flattened (step, batch) columns
NCORES = 8
GS = 4          # steps per x-side matmul group (max; group 0 is 1 step)
GROUPS = [(0, 1)] + [(g0, min(GS, S - g0)) for g0 in range(1, S, GS)]
NG = len(GROUPS)  # 14: sizes 1, 4x12, 1

CORE_PH = [(0, 0), (0, 1), (1, 0), (1, 1), (2, 0), (2, 1), (0, 0), (0, 1)]

_BF = ml_dtypes.bfloat16

# scoring chunking (unchanged from baseline)
T_PAD = 6272            # 49 * 128, per-core padded item count
N_CHUNK = 512
CHUNK_SIZES = [512] * 12 + [128]
CHUNK_OFFS = np.cumsum([0] + CHUNK_SIZES).tolist()


# --------------------------------------------------------------------------
# Launch 1: scan
# --------------------------------------------------------------------------

# Planned-phase schedule (ns, tuned against TimelineSim).
P = 2210            # target steady-state period per step
OFF_B = 560         # chain B phase offset
HEAD = 4200         # planned time of step 0 merged sigmoid
# per-chain op phases relative to base(t) = HEAD + t*P (+OFF_B for chain B)
_PHC = dict(sr=0, u1=510, u2=862, a=538, q=822, tanh=1070, hq=1262,
            an=1526, hnew=1742, rmm=-473, nmm=-259)
PH = [_PHC, _PHC]
XPH = 560           # x-matmul window start within a period

EMIT_LOG = {}       # instruction name -> (label, planned ns)


def build_scan_nc():
    nc = bacc.Bacc("TRN2", target_bir_lowering=False, debug=False,
                   num_devices=NCORES)

    # weight layout: [128 k-part, 2 k-tile, 768 gate-cols]; gate cols are
    # [r(0:256) | i(256:512) | n(512:768)]
    HB = 5 * BH     # leading x/cf columns staged in the head blob (steps 0-4)
    # head blob per partition: [wiT-r (2*256) | x0 (2*HB) | cf0 (HB)]
    HBW = 2 * 256 + 2 * HB + HB
    blob_d = nc.dram_tensor("blob", [128, HBW], BF16, kind="ExternalInput")
    bias_d = nc.dram_tensor("bias", [1, 1024], BF16, kind="ExternalInput")
    wiT_d = nc.dram_tensor("wiT_in", [128, 2, 512], BF16,
                           kind="ExternalInput")
    whT_d = nc.dram_tensor("whT", [128, 2, 768], BF16, kind="ExternalInput")
    xT_d = nc.dram_tensor("xT", [128, 2, SB], BF16, kind="ExternalInput")
    cf_d = nc.dram_tensor("cf_lin", [SB // 128, 128], BF16,
                          kind="ExternalInput")
    hn_parts = nc.dram_tensor("hn_parts", [128, 2, 2, BH], BF16,
                              kind="ExternalOutput")
    weT_d = [nc.dram_tensor(f"weT{p}", [128, 2, T_PAD], BF16,
                            kind="ExternalInput") for p in range(3)]

    events = []

    def at(tns, fn, label=None):
        events.append((tns, len(events), fn, label))

    with tile.TileContext(nc) as tc:
        with (
            tc.tile_pool(name="wetabp", bufs=1, side="right") as wetabp,
            tc.tile_pool(name="consts", bufs=1) as consts,
            tc.tile_pool(name="psumc", bufs=1, space="PSUM") as psumc,
            tc.tile_pool(name="ginp", bufs=2) as ginp,
            tc.tile_pool(name="ew", bufs=3) as ew,
            tc.tile_pool(name="hpool", bufs=3) as hpool,
        ):
            st = {}   # emitted-tile state

            def mk_wetab():
                # persistent right-side table region, reused by the score
                # launch at the same SBUF address
                st['wetab'] = wetabp.tile([128, 3, 2, T_PAD], BF16,
                                          name='wetab')
            at(-2100, mk_wetab)

            def tab_dma(p):
                nc.sync.dma_start(st['wetab'][:, p], weT_d[p].ap())
            for p in range(3):
                at(HEAD + (4 + 2 * p) * P, lambda p=p: tab_dma(p))

            # ---------------- input staging ----------------
            def head_dmas():
                # bias first (tiny), then one blob DMA with everything the
                # first five steps need, then the bulk weights
                st['bias'] = consts.tile([1, 1024], BF16, name='bias')
                nc.sync.dma_start(st['bias'][:], bias_d.ap())
                st['blob'] = consts.tile([128, HBW], BF16, name='blob')
                nc.sync.dma_start(st['blob'][:], blob_d.ap())
                st['wiTr'] = st['blob'][:, ds(0, 512)].rearrange(
                    "p (k c) -> p k c", k=2)
                st['x0'] = st['blob'][:, ds(512, 2 * HB)].rearrange(
                    "p (k c) -> p k c", k=2)
                st['cf0'] = st['blob'][:, ds(512 + 2 * HB, HB)]
                st['xT'] = consts.tile([128, 2, SB], BF16, name='xT')
                st['cf'] = consts.tile([128, SB], BF16, name='cf')
                cf_flat = cf_d.ap().rearrange("t p -> (t p)")
                st['cf_flat'] = cf_flat
                st['wiT'] = consts.tile([128, 2, 512], BF16, name='wiT')
                nc.sync.dma_start(st['wiT'][:], wiT_d.ap())
                st['whT'] = consts.tile([128, 2, 768], BF16, name='whT')
                nc.sync.dma_start(st['whT'][:], whT_d.ap())
                st['ones'] = consts.tile([1, GS * BH], BF16, name='ones')
                nc.vector.memset(st['ones'][:], 1.0)
                for s_ in range(2):
                    st[f'bkri{s_}'] = psumc.tile(
                        [128, 2, 4, GS, SW], F32, name=f'bkri{s_}')
                    st[f'bkn{s_}'] = psumc.tile(
                        [128, 2, 2, GS, SW], F32, name=f'bkn{s_}')
                    st[f'bkghn{s_}'] = psumc.tile(
                        [128, 2, SW], F32, name=f'bkghn{s_}')
                for s_ in range(2):
                    hs = hpool.tile([128, 2, SW], BF16, tag=f"h{s_}",
                                    name=f"h_init{s_}")
                    nc.vector.memset(hs[:], 0.0)
                    st[f'h{s_}'] = hs
            at(-2000, head_dmas)

            # streamed x/cf: 2 chunks each, issued right behind the head
            def tail_dmas(qf):
                xq = (SB - HB) // 2
                nc.sync.dma_start(
                    st['xT'][:, :, ds(HB + qf * xq, xq)],
                    xT_d.ap()[:, :, ds(HB + qf * xq, xq)])
                nc.sync.dma_start(
                    st['cf'][:, ds(HB + qf * xq, xq)],
                    st['cf_flat'][None, ds(HB + qf * xq, xq)]
                    .to_broadcast((128, xq)))
            for qf in range(2):
                at(HEAD + qf * 2 * P, lambda qf=qf: tail_dmas(qf))

            # ---------------- x-side groups ----------------
            def mk_group_tiles(gi, s_):
                b = gi % 2
                st[f'pri{b}_{s_}'] = st[f'bkri{s_}'][:, b]
                st[f'pn{b}_{s_}'] = st[f'bkn{s_}'][:, b]

            def xmm(gi, s_, j, k):
                # j in 0..5 (r0 r1 i0 i1 n0 n1); k: None=bias seed else ktile
                g0, gn = GROUPS[gi]
                if j < 4:
                    out = st[f'pri{gi % 2}_{s_}'][:, j, :gn, :]
                else:
                    out = st[f'pn{gi % 2}_{s_}'][:, j - 4, :gn, :]
                if k is None:
                    nc.tensor.matmul(out, st['bias'][0:1, ts(j, 128)],
                                     st['ones'][0:1, ds(0, gn * SW)],
                                     start=True, stop=False)
                    return
                if j < 2:
                    w = st['wiTr'][:, k, ts(j, 128)]
                else:
                    w = st['wiT'][:, k, ds((j - 2) * 128, 128)]
                if g0 + gn <= 5:
                    xsrc = st['x0'][:, k, ds(g0 * BH, gn * BH)]
                else:
                    xsrc = st['xT'][:, k, ds(g0 * BH, gn * BH)]
                nc.tensor.matmul(
                    out, w,
                    xsrc.rearrange("p (t b) -> p t b", b=BH)
                    [:, :, ds(s_ * SW, SW)],
                    start=False, stop=(k == 1))

            def gin_copy(gi, s_, half=None):
                g0, gn = GROUPS[gi]
                if half is None or half == 0:
                    gsb = ginp.tile([128, 2, GS, SW], BF16, tag=f"gin{s_}",
                                    name=f"gin_sb{s_}")
                    st[f'gin_new{s_}'] = gsb
                gsb = st[f'gin_new{s_}']
                st[f'gin_g{gi}_{s_}'] = gsb
                pn = st[f'pn{gi % 2}_{s_}']
                if half is None:
                    nc.scalar.copy(gsb[:, :, :gn, :], pn[:, :, :gn, :])
                elif s_ == 0:
                    nc.scalar.copy(gsb[:, half, :gn, :], pn[:, half, :gn, :])
                else:
                    nc.vector.tensor_copy(gsb[:, half, :gn, :],
                                          pn[:, half, :gn, :])

            # group 0: in the head; group g>=1: spread over group g-1
            for gi in range(NG):
                g0, gn = GROUPS[gi]
                if gi == 0:
                    t0 = HEAD - 650
                    for s_ in range(2):
                        at(t0 - 10, lambda gi=gi, s_=s_: mk_group_tiles(gi, s_))
                    idx = 0
                    for s_ in range(2):
                        for j in range(6):
                            at(t0 + idx * 55,
                               lambda gi=gi, s_=s_, j=j: xmm(gi, s_, j, None))
                            for k in range(2):
                                at(t0 + idx * 55 + 18 + 18 * k,
                                   lambda gi=gi, s_=s_, j=j, k=k:
                                   xmm(gi, s_, j, k))
                            idx += 1
                        at(t0 + idx * 55 + 40,
                           lambda gi=gi, s_=s_: gin_copy(gi, s_, None))
                else:
                    # spread 36 matmuls over the periods of group gi-1,
                    # n-gate blocks first so gin copies can start early
                    p0, pn_ = GROUPS[gi - 1]
                    nper = GROUPS[gi - 1][1]
                    base0 = HEAD + p0 * P
                    at(base0 + XPH - 20,
                       lambda gi=gi: (mk_group_tiles(gi, 0),
                                      mk_group_tiles(gi, 1)))
                    jorder = [(0, 4), (0, 5), (1, 4), (1, 5),
                              (0, 0), (0, 1), (0, 2), (0, 3),
                              (1, 0), (1, 1), (1, 2), (1, 3)]
                    for idx, (s_, j) in enumerate(jorder):
                        per = min(idx // 3, nper - 1)
                        slot = idx % 3
                        tt = base0 + per * P + XPH + slot * 200
                        at(tt, lambda gi=gi, s_=s_, j=j: xmm(gi, s_, j, None))
                        for k in range(2):
                            at(tt + 55 + 55 * k,
                               lambda gi=gi, s_=s_, j=j, k=k:
                               xmm(gi, s_, j, k))
                    # chain-0 gin halves on Act (phase 1310 of periods 1,2);
                    # chain-1 halves on DVE (phase 120 of periods 2,3)
                    mkt = base0 + XPH - 20
                    for hf in range(2):
                        at(max(base0 + min(1 + hf, nper - 1) * P + 1310,
                               mkt + 1500 + hf),
                           lambda gi=gi, hf=hf: gin_copy(gi, 0, hf))
                        at(max(base0 + min(2 + hf, nper - 1) * P + 120,
                               mkt + 1500 + 2 + hf),
                           lambda gi=gi, hf=hf: gin_copy(gi, 1, hf))

            # ---------------- per-step chain ----------------
            def hmms(t, s_, which):
                gi = t // GS
                tl = t % GS
                whT = st['whT']
                hqv = st.get(f'hq{s_}')
                anv = st.get(f'an{s_}')
                if which == 'r':
                    pri = st[f'pri{gi % 2}_{s_}']
                    for j in range(4):
                        for hx in (hqv, anv):
                            for k in range(2):
                                nc.tensor.matmul(
                                    pri[:, j, tl, :], whT[:, k, ts(j, 128)],
                                    hx[:, k, :], start=False, stop=False,
                                    skip_group_check=True)
                elif which == 'n':
                    ps = st[f'bkghn{s_}']
                    st[f'ghn{s_}'] = ps
                    for j in range(2):
                        nc.tensor.matmul(
                            ps[:, j, :], st['bias'][0:1, ts(6 + j, 128)],
                            st['ones'][0:1, ds(0, SW)], start=True,
                            stop=(t == 0))
                        if t > 0:
                            for hi, hx in enumerate((hqv, anv)):
                                for k in range(2):
                                    nc.tensor.matmul(
                                        ps[:, j, :],
                                        whT[:, k, ds(512 + j * 128, 128)],
                                        hx[:, k, :], start=False,
                                        stop=(hi == 1 and k == 1))

            def op_sr(t, s_):
                gi, tl = t // GS, t % GS
                ri_sb = ew.tile([128, 4, SW], BF16, tag=f"ri{s_}",
                                name=f"ri_sb{s_}")
                nc.scalar.activation(ri_sb[:],
                                     st[f'pri{gi % 2}_{s_}'][:, :, tl, :],
                                     AF.Sigmoid)
                st[f'r{s_}'] = ri_sb[:, 0:2, :]
                st[f'i{s_}'] = ri_sb[:, 2:4, :]

            def op_u1(t, s_):
                u1 = ew.tile([128, 2, SW], BF16, tag=f"u1{s_}",
                             name=f"u1_{s_}")
                nc.vector.tensor_tensor(u1[:], st[f'ghn{s_}'][:],
                                        st[f'r{s_}'], OP.mult)
                st[f'u1{s_}'] = u1

            def op_u2(t, s_):
                gi, tl = t // GS, t % GS
                u2 = ew.tile([128, 2, SW], BF16, tag=f"u2{s_}",
                             name=f"u2_{s_}")
                nc.vector.tensor_tensor(u2[:], st[f'u1{s_}'][:],
                                        st[f'gin_g{gi}_{s_}'][:, :, tl, :],
                                        OP.add)
                st[f'u2{s_}'] = u2

            def op_tanh(t, s_):
                n_t = ew.tile([128, 2, SW], BF16, tag=f"n{s_}",
                              name=f"n_t{s_}")
                nc.scalar.activation(n_t[:], st[f'u2{s_}'][:], AF.Tanh)
                st[f'n{s_}'] = n_t

            def op_a(t, s_):
                a_t = ew.tile([128, 2, SW], BF16, tag=f"a{s_}",
                              name=f"a_t{s_}")
                cfsrc = (st['cf0'] if t < 5 else st['cf'])
                cfoff = t * BH + s_ * SW
                nc.gpsimd.tensor_tensor(
                    a_t[:], st[f'i{s_}'],
                    cfsrc[:, None, ds(cfoff, SW)]
                    .to_broadcast((128, 2, SW)), OP.mult)
                st[f'a{s_}'] = a_t

            def op_q(t, s_):
                q_t = ew.tile([128, 2, SW], BF16, tag=f"q{s_}",
                              name=f"q_t{s_}")
                nc.gpsimd.tensor_scalar(q_t[:], st[f'a{s_}'][:], -1.0, 1.0,
                                        OP.mult, OP.add)
                st[f'q{s_}'] = q_t

            def op_hq(t, s_):
                hq = hpool.tile([128, 2, SW], BF16, tag=f"hq{s_}",
                                name=f"hq{s_}")
                nc.vector.tensor_tensor(hq[:], st[f'h{s_}'][:],
                                        st[f'q{s_}'][:], OP.mult)
                st[f'hq{s_}'] = hq

            def op_an(t, s_):
                an = hpool.tile([128, 2, SW], BF16, tag=f"an{s_}",
                                name=f"an{s_}")
                nc.vector.tensor_tensor(an[:], st[f'a{s_}'][:],
                                        st[f'n{s_}'][:], OP.mult)
                st[f'an{s_}'] = an

            def op_hnew(t, s_):
                h_new = hpool.tile([128, 2, SW], BF16, tag=f"h{s_}",
                                   name=f"h_new{s_}")
                eng = nc.vector if t == S - 1 else nc.gpsimd
                eng.tensor_tensor(h_new[:], st[f'hq{s_}'][:],
                                  st[f'an{s_}'][:], OP.add)
                st[f'h{s_}'] = h_new

            for t in range(S):
                for s_ in range(2):
                    ph = PH[s_]
                    base = HEAD + t * P + (OFF_B if s_ else 0)
                    if t > 0:
                        at(base + ph['rmm'], lambda t=t, s_=s_: hmms(t, s_, 'r'),
                           f"rmm.{s_}.{t}")
                    at(base + ph['nmm'], lambda t=t, s_=s_: hmms(t, s_, 'n'),
                       f"nmm.{s_}.{t}")
                    at(base + ph['sr'], lambda t=t, s_=s_: op_sr(t, s_),
                       f"sr.{s_}.{t}")
                    at(base + ph['u1'], lambda t=t, s_=s_: op_u1(t, s_),
                       f"u1.{s_}.{t}")
                    at(base + ph['u2'], lambda t=t, s_=s_: op_u2(t, s_),
                       f"u2.{s_}.{t}")
                    at(base + ph['a'], lambda t=t, s_=s_: op_a(t, s_),
                       f"a.{s_}.{t}")
                    at(base + ph['q'], lambda t=t, s_=s_: op_q(t, s_),
                       f"q.{s_}.{t}")
                    at(base + ph['tanh'], lambda t=t, s_=s_: op_tanh(t, s_),
                       f"tanh.{s_}.{t}")
                    at(base + ph['hq'], lambda t=t, s_=s_: op_hq(t, s_),
                       f"hq.{s_}.{t}")
                    at(base + ph['an'], lambda t=t, s_=s_: op_an(t, s_),
                       f"an.{s_}.{t}")
                    at(base + ph['hnew'], lambda t=t, s_=s_: op_hnew(t, s_),
                       f"hnew.{s_}.{t}")

            def final_dma(s_):
                eng = nc.sync if s_ == 0 else nc.scalar
                eng.dma_start(hn_parts.ap()[:, 0, :, ds(s_ * SW, SW)],
                              st[f'h{s_}'][:])
            for s_ in range(2):
                base49 = HEAD + (S - 1) * P + (OFF_B if s_ else 0)
                at(base49 + PH[s_]['hnew'] + 60,
                   lambda s_=s_: final_dma(s_))

            # ---- emit in planned order ----
            events.sort(key=lambda e: (e[0], e[1]))
            for tns, _, fn, label in events:
                n0 = len(nc.inst_map)
                with tc.tile_wait_until(max(tns, 0) / 1e6):
                    fn()
                if label is not None:
                    for iname in list(nc.inst_map)[n0:]:
                        EMIT_LOG[iname] = (label, tns)

    nc.compile()
    return nc


def scan_host_inputs(seq, emb, emb_purpose, w_ih, w_hh, b_ih, b_hh):
    seq = np.asarray(seq)
    xg = emb[seq]                      # [S, B, D] gather (input staging)
    cs = np.einsum("sbd,pd->sbp", xg, emb_purpose) / TAU
    cs -= cs.max(axis=2, keepdims=True)
    ce = np.exp(cs)
    cw_full = ce / ce.sum(axis=2, keepdims=True)     # [S, B, 3]
    cw_full *= (seq != 0)[:, :, None]
    cw_full *= (cw_full >= EPS)
    HB = 5 * BH
    in_maps = []
    for c in range(NCORES):
        p, hh = CORE_PH[c]
        sl = slice(hh * BH, (hh + 1) * BH)
        xh = xg[:, sl, :]              # [S, BH, D]
        xT = np.ascontiguousarray(
            xh.transpose(2, 0, 1).reshape(2, 128, SB).transpose(1, 0, 2))
        wiT = np.ascontiguousarray(
            w_ih[p].T.reshape(2, 128, 768).transpose(1, 0, 2))
        whT = np.ascontiguousarray(
            w_hh[p].T.reshape(2, 128, 768).transpose(1, 0, 2))
        cf = cw_full[:, sl, p].reshape(SB)
        bias = np.concatenate([
            (b_ih[p] + b_hh[p])[:512],      # r, i combined biases
            b_ih[p][512:],                   # n input bias
            b_hh[p][512:],                   # n hidden bias
        ])
        # blob: [wiT r-rows (2,256) | x cols 0:HB (2,HB) | cf 0:HB bcast]
        blob = np.concatenate([
            wiT[:, :, 0:256].reshape(128, 512),
            xT[:, :, 0:HB].reshape(128, 2 * HB),
            np.broadcast_to(cf[None, 0:HB], (128, HB)),
        ], axis=1)
        in_maps.append({
            "blob": blob.astype(_BF),
            "wiT_in": np.ascontiguousarray(wiT[:, :, 256:768]).astype(_BF),
            "whT": whT.astype(_BF),
            "xT": xT.astype(_BF),
            "cf_lin": cf.reshape(SB // 128, 128).astype(_BF),
            "bias": bias[None, :].astype(_BF),
        })
    return in_maps


# --------------------------------------------------------------------------
# Launch 2: scoring (baseline structure; DMA slice order fixed so each
# q-range's three purposes arrive together)
# --------------------------------------------------------------------------

def build_score_nc():
    nc = bacc.Bacc("TRN2", target_bir_lowering=False, debug=False,
                   num_devices=NCORES)

    hT6_d = nc.dram_tensor("hT6", [128, 6, 128], BF16, kind="ExternalInput")
    scores_d = nc.dram_tensor("scores", [128, T_PAD], BF16,
                              kind="ExternalOutput")

    with tile.TileContext(nc) as tc:
        with (
            tc.tile_pool(name="wetabp", bufs=1, side="right") as wetabp,
            tc.tile_pool(name="consts", bufs=1) as consts,
            tc.tile_pool(name="epsum", bufs=2, space="PSUM") as epsum,
            tc.tile_pool(name="outp", bufs=1) as outp,
        ):
            # same right-side allocation as the scan launch: the tables are
            # already resident in SBUF from launch 1. The pad-column memset
            # marks the tile initialized for dep tracking without touching
            # table data (cols >= 6251 are padding).
            wetab = wetabp.tile([128, 3, 2, T_PAD], BF16, name='wetab')
            nc.vector.memset(wetab[:, 0, 0, ds(T_PAD - 1, 1)], 0.0)
            hT6 = consts.tile([128, 6, 128], BF16)
            nc.sync.dma_start(hT6[:], hT6_d.ap())

            out_sb = outp.tile([128, T_PAD], BF16)
            NCH = len(CHUNK_SIZES)
            for ci, (c0, cs) in enumerate(zip(CHUNK_OFFS[:-1], CHUNK_SIZES)):
                ps = epsum.tile([128, N_CHUNK], F32, tag=f"S{ci % 2}",
                                name="ps_s")
                for p in range(3):
                    for k in range(2):
                        nc.tensor.matmul(
                            ps[:, :cs], hT6[:, p * 2 + k, :],
                            wetab[:, p, k, ds(c0, cs)],
                            start=(p == 0 and k == 0),
                            stop=(p == 2 and k == 1))
                if ci >= NCH - 2:
                    # tail chunks: copy on DVE (off the busy Act queue) and
                    # write back each chunk on its own queue immediately
                    nc.vector.tensor_copy(out_sb[:, ds(c0, cs)], ps[:, :cs])
                    nc.sync.dma_start(scores_d.ap()[:, ds(c0, cs)],
                                      out_sb[:, ds(c0, cs)])
                else:
                    nc.scalar.copy(out_sb[:, ds(c0, cs)], ps[:, :cs])
            WB = [(0, 2), (2, 3), (5, 3), (8, 3)]
            for b0, bn in WB:
                o0 = CHUNK_OFFS[b0]
                o1 = CHUNK_OFFS[b0 + bn]
                nc.scalar.dma_start(scores_d.ap()[:, ds(o0, o1 - o0)],
                                    out_sb[:, ds(o0, o1 - o0)])

    nc.compile()
    return nc


def score_table_inputs(emb, emb_purpose):
    lg = emb @ emb_purpose.T                   # [T, 3]
    e = np.exp(lg - lg.max(axis=1, keepdims=True))
    tcw = (e / e.sum(axis=1, keepdims=True)).astype(np.float32)  # [T, 3]

    base = N_ITEMS // NCORES
    rem = N_ITEMS - base * NCORES
    bounds = []
    s0 = 0
    for c in range(NCORES):
        n = base + (1 if c < rem else 0)
        bounds.append((s0, s0 + n))
        s0 += n

    in_maps = []
    for c in range(NCORES):
        lo, hi = bounds[c]
        n = hi - lo
        m = {}
        for p in range(3):
            we = (emb[lo:hi] * tcw[lo:hi, p:p + 1]).T.astype(_BF)  # [256, n]
            weT = np.zeros((128, 2, T_PAD), _BF)
            weT[:, :, :n] = we.reshape(2, 128, n).transpose(1, 0, 2)
            m[f"weT{p}"] = weT
        in_maps.append(m)
    return in_maps, bounds


# --------------------------------------------------------------------------
# Entry point
# --------------------------------------------------------------------------

_SCAN_NC = None
_SCORE_NC = None


def _wetab_addr(nc):
    for alloc in nc.m.functions[0].allocations:
        if "wetab" in str(getattr(alloc, "name", "")):
            return alloc.memorylocations[0].addr
    raise RuntimeError("wetab allocation not found")


def _get_ncs():
    global _SCAN_NC, _SCORE_NC
    if _SCAN_NC is None:
        _SCAN_NC = build_scan_nc()
    if _SCORE_NC is None:
        _SCORE_NC = build_score_nc()
    assert _wetab_addr(_SCAN_NC) == _wetab_addr(_SCORE_NC), (
        "persistent table region addresses diverged between launches")
    return _SCAN_NC, _SCORE_NC


def kernel(seq, emb, emb_purpose, w_ih, w_hh, b_ih, b_hh):
    seq = np.asarray(seq)
    emb = np.asarray(emb, np.float32)
    emb_purpose = np.asarray(emb_purpose, np.float32)
    w_ih = np.asarray(w_ih, np.float32)
    w_hh = np.asarray(w_hh, np.float32)
    b_ih = np.asarray(b_ih, np.float32)
    b_hh = np.asarray(b_hh, np.float32)

    scan_nc, score_nc = _get_ncs()

    scan_ins = scan_host_inputs(seq, emb, emb_purpose, w_ih, w_hh, b_ih, b_hh)
    tab_ins, bounds = score_table_inputs(emb, emb_purpose)
    for m, t in zip(scan_ins, tab_ins):
        m.update(t)
    res1 = run_bass_kernel_spmd(scan_nc, scan_ins, core_ids=list(range(NCORES)))

    hT6 = np.zeros((128, 6, 128), _BF)
    for c in range(6):
        p, hh = CORE_PH[c]
        sl = res1.results[c]["hn_parts"][:, 0]   # [128, 2, BH]
        for k in range(2):
            hT6[:, p * 2 + k, hh * BH:(hh + 1) * BH] = sl[:, k, :]

    score_ins = [{"hT6": hT6}] * NCORES
    res2 = run_bass_kernel_spmd(score_nc, score_ins, core_ids=list(range(NCORES)))

    scores = np.empty((B, N_ITEMS), np.float32)
    for c in range(NCORES):
        lo, hi = bounds[c]
        scores[:, lo:hi] = res2.results[c]["scores"][:, : hi - lo]\
            .astype(np.float32)
    return scores


# revision 6
# speedup vs baseline: 1.0058x; 1.0058x over previous
"""Trainium2 Bass kernel for MCPRN (purpose-routed GRU-variant recommender).

Two SPMD launches on 8 NeuronCores, with a persistent-SBUF bridge:

Launch 1 (scan): six cores run (purpose p, batch-half h) PSRU scans as two
anti-phased B=32 sub-chains, statically software-pipelined at ~2.14us/step
(planned-phase emission; the tile scheduler follows the planned order):
  PE r-gate h-matmuls -> sigmoid(r) [Act] -> u1 = ghn*r [DVE] ->
  u2 = u1+gin [DVE] -> tanh [Act] -> an = a*n [DVE] -> next step's matmuls.
  sigmoid(i), a = cf*i, q = 1-a, h*q, h updates, and the x-side group
  matmuls all run off the critical chain in engine idle windows.
  Hidden state is kept split (h = hq + an) so the recurrent matmuls take
  hq/an as two accumulating moving operands.
  While the scan runs (latency-bound, DMA idle), the three softmax-
  pre-weighted scoring tables we_p = emb * tcw[:,p] (9.6 MB/core) stream
  into a right-side SBUF region that persists across launches.

Launch 2 (score): each core scores its ~6250-item slice reading the
pre-staged tables directly from persistent SBUF (verified same address at
build time) -- no table DMA. Six matmuls per 512-item chunk accumulate in
one PSUM bank; PSUM->SBUF copies on Act/DVE; chunked writebacks.

Host work is input staging only: emb[seq] gather, concen-softmax weights,
tcw table pre-weighting, and the final unshard/concat.
"""

import numpy as np
import ml_dtypes

import concourse.bacc as bacc
import concourse.mybir as mybir
import concourse.tile as tile
from concourse.bass import ts, ds
from concourse.bass_utils import run_bass_kernel_spmd

F32 = mybir.dt.float32
BF16 = mybir.dt.bfloat16
AF = mybir.ActivationFunctionType
OP = mybir.AluOpType

N_ITEMS = 50001
DIM = 256
TAU = 0.1
S = 50
B = 128
EPS = 0.01
BH = 64         # batch per scan core
SW = 32         # batch per sub-chain
SB = S * BH     # 3200 flattened (step, batch) columns
NCORES = 8
GS = 4          # steps per x-side matmul group (max; group 0 is 1 step)
GROUPS = [(0, 1)] + [(g0, min(GS, S - g0)) for g0 in range(1, S, GS)]
NG = len(GROUPS)  # 14: sizes 1, 4x12, 1

CORE_PH = [(0, 0), (0, 1), (1, 0), (1, 1), (2, 0), (2, 1), (0, 0), (0, 1)]

_BF = ml_dtypes.bfloat16

# scoring chunking (unchanged from baseline)
T_PAD = 6272            # 49 * 128, per-core padded item count
N_CHUNK = 512
CHUNK_SIZES = [512] * 12 + [128]
CHUNK_OFFS = np.cumsum([0] + CHUNK_SIZES).tolist()


# --------------------------------------------------------------------------
# Launch 1: scan
# --------------------------------------------------------------------------

# Planned-phase schedule (ns, tuned against TimelineSim).
P = 2100            # target steady-state period per step
OFF_B = 510         # chain B phase offset
HEAD = 4200         # planned time of step 0 merged sigmoid
# per-chain op phases relative to base(t) = HEAD + t*P (+OFF_B for chain B)
_PHC = dict(sr=0, u1=510, u2=862, a=538, q=822, tanh=1070, hq=1262,
            an=1526, hnew=1742, rmm=-473, nmm=-259)
PH = [_PHC, _PHC]
XPH = 560           # x-matmul window start within a period

EMIT_LOG = {}       # instruction name -> (label, planned ns)


def build_scan_nc():
    nc = bacc.Bacc("TRN2", target_bir_lowering=False, debug=False,
                   num_devices=NCORES)

    # weight layout: [128 k-part, 2 k-tile, 768 gate-cols]; gate cols are
    # [r(0:256) | i(256:512) | n(512:768)]
    HB = 5 * BH     # leading x/cf columns staged in the head blob (steps 0-4)
    # head blob per partition: [wiT-r (2*256) | x0 (2*HB) | cf0 (HB)]
    HBW = 2 * 256 + 2 * HB + HB
    blob_d = nc.dram_tensor("blob", [128, HBW], BF16, kind="ExternalInput")
    bias_d = nc.dram_tensor("bias", [1, 1024], BF16, kind="ExternalInput")
    wiT_d = nc.dram_tensor("wiT_in", [128, 2, 512], BF16,
                           kind="ExternalInput")
    whT_d = nc.dram_tensor("whT", [128, 2, 768], BF16, kind="ExternalInput")
    xT_d = nc.dram_tensor("xT", [128, 2, SB], BF16, kind="ExternalInput")
    cf_d = nc.dram_tensor("cf_lin", [SB // 128, 128], BF16,
                          kind="ExternalInput")
    hn_parts = nc.dram_tensor("hn_parts", [128, 2, 2, BH], BF16,
                              kind="ExternalOutput")
    weT_d = [nc.dram_tensor(f"weT{p}", [128, 2, T_PAD], BF16,
                            kind="ExternalInput") for p in range(3)]

    events = []

    def at(tns, fn, label=None):
        events.append((tns, len(events), fn, label))

    with tile.TileContext(nc) as tc:
        with (
            tc.tile_pool(name="wetabp", bufs=1, side="right") as wetabp,
            tc.tile_pool(name="consts", bufs=1) as consts,
            tc.tile_pool(name="psumc", bufs=1, space="PSUM") as psumc,
            tc.tile_pool(name="ginp", bufs=2) as ginp,
            tc.tile_pool(name="ew", bufs=3) as ew,
            tc.tile_pool(name="hpool", bufs=3) as hpool,
        ):
            st = {}   # emitted-tile state

            def mk_wetab():
                # persistent right-side table region, reused by the score
                # launch at the same SBUF address
                st['wetab'] = wetabp.tile([128, 3, 2, T_PAD], BF16,
                                          name='wetab')
            at(-2100, mk_wetab)

            def tab_dma(p):
                nc.sync.dma_start(st['wetab'][:, p], weT_d[p].ap())
            for p in range(3):
                at(HEAD + (4 + 2 * p) * P, lambda p=p: tab_dma(p))

            # ---------------- input staging ----------------
            def head_dmas():
                # bias first (tiny), then one blob DMA with everything the
                # first five steps need, then the bulk weights
                st['bias'] = consts.tile([1, 1024], BF16, name='bias')
                nc.sync.dma_start(st['bias'][:], bias_d.ap())
                st['blob'] = consts.tile([128, HBW], BF16, name='blob')
                nc.sync.dma_start(st['blob'][:], blob_d.ap())
                st['wiTr'] = st['blob'][:, ds(0, 512)].rearrange(
                    "p (k c) -> p k c", k=2)
                st['x0'] = st['blob'][:, ds(512, 2 * HB)].rearrange(
                    "p (k c) -> p k c", k=2)
                st['cf0'] = st['blob'][:, ds(512 + 2 * HB, HB)]
                st['xT'] = consts.tile([128, 2, SB], BF16, name='xT')
                st['cf'] = consts.tile([128, SB], BF16, name='cf')
                cf_flat = cf_d.ap().rearrange("t p -> (t p)")
                st['cf_flat'] = cf_flat
                st['wiT'] = consts.tile([128, 2, 512], BF16, name='wiT')
                nc.sync.dma_start(st['wiT'][:], wiT_d.ap())
                st['whT'] = consts.tile([128, 2, 768], BF16, name='whT')
                nc.sync.dma_start(st['whT'][:], whT_d.ap())
                st['ones'] = consts.tile([1, GS * BH], BF16, name='ones')
                nc.vector.memset(st['ones'][:], 1.0)
                for s_ in range(2):
                    st[f'bkri{s_}'] = psumc.tile(
                        [128, 2, 4, GS, SW], F32, name=f'bkri{s_}')
                    st[f'bkn{s_}'] = psumc.tile(
                        [128, 2, 2, GS, SW], F32, name=f'bkn{s_}')
                    st[f'bkghn{s_}'] = psumc.tile(
                        [128, 2, SW], F32, name=f'bkghn{s_}')
                for s_ in range(2):
                    hs = hpool.tile([128, 2, SW], BF16, tag=f"h{s_}",
                                    name=f"h_init{s_}")
                    nc.vector.memset(hs[:], 0.0)
                    st[f'h{s_}'] = hs
            at(-2000, head_dmas)

            # streamed x/cf: 2 chunks each, issued right behind the head
            def tail_dmas(qf):
                xq = (SB - HB) // 2
                nc.sync.dma_start(
                    st['xT'][:, :, ds(HB + qf * xq, xq)],
                    xT_d.ap()[:, :, ds(HB + qf * xq, xq)])
                nc.sync.dma_start(
                    st['cf'][:, ds(HB + qf * xq, xq)],
                    st['cf_flat'][None, ds(HB + qf * xq, xq)]
                    .to_broadcast((128, xq)))
            for qf in range(2):
                at(HEAD + qf * 2 * P, lambda qf=qf: tail_dmas(qf))

            # ---------------- x-side groups ----------------
            def mk_group_tiles(gi, s_):
                b = gi % 2
                st[f'pri{b}_{s_}'] = st[f'bkri{s_}'][:, b]
                st[f'pn{b}_{s_}'] = st[f'bkn{s_}'][:, b]

            def xmm(gi, s_, j, k):
                # j in 0..5 (r0 r1 i0 i1 n0 n1); k: None=bias seed else ktile
                g0, gn = GROUPS[gi]
                if j < 4:
                    out = st[f'pri{gi % 2}_{s_}'][:, j, :gn, :]
                else:
                    out = st[f'pn{gi % 2}_{s_}'][:, j - 4, :gn, :]
                if k is None:
                    nc.tensor.matmul(out, st['bias'][0:1, ts(j, 128)],
                                     st['ones'][0:1, ds(0, gn * SW)],
                                     start=True, stop=False)
                    return
                if j < 2:
                    w = st['wiTr'][:, k, ts(j, 128)]
                else:
                    w = st['wiT'][:, k, ds((j - 2) * 128, 128)]
                if g0 + gn <= 5:
                    xsrc = st['x0'][:, k, ds(g0 * BH, gn * BH)]
                else:
                    xsrc = st['xT'][:, k, ds(g0 * BH, gn * BH)]
                nc.tensor.matmul(
                    out, w,
                    xsrc.rearrange("p (t b) -> p t b", b=BH)
                    [:, :, ds(s_ * SW, SW)],
                    start=False, stop=(k == 1))

            def gin_copy(gi, s_, half=None):
                g0, gn = GROUPS[gi]
                if half is None or half == 0:
                    gsb = ginp.tile([128, 2, GS, SW], BF16, tag=f"gin{s_}",
                                    name=f"gin_sb{s_}")
                    st[f'gin_new{s_}'] = gsb
                gsb = st[f'gin_new{s_}']
                st[f'gin_g{gi}_{s_}'] = gsb
                pn = st[f'pn{gi % 2}_{s_}']
                if half is None:
                    nc.scalar.copy(gsb[:, :, :gn, :], pn[:, :, :gn, :])
                elif s_ == 0:
                    nc.scalar.copy(gsb[:, half, :gn, :], pn[:, half, :gn, :])
                else:
                    nc.vector.tensor_copy(gsb[:, half, :gn, :],
                                          pn[:, half, :gn, :])

            # group 0: in the head; group g>=1: spread over group g-1
            for gi in range(NG):
                g0, gn = GROUPS[gi]
                if gi == 0:
                    t0 = HEAD - 650
                    for s_ in range(2):
                        at(t0 - 10, lambda gi=gi, s_=s_: mk_group_tiles(gi, s_))
                    idx = 0
                    for s_ in range(2):
                        for j in range(6):
                            at(t0 + idx * 55,
                               lambda gi=gi, s_=s_, j=j: xmm(gi, s_, j, None))
                            for k in range(2):
                                at(t0 + idx * 55 + 18 + 18 * k,
                                   lambda gi=gi, s_=s_, j=j, k=k:
                                   xmm(gi, s_, j, k))
                            idx += 1
                        at(t0 + idx * 55 + 40,
                           lambda gi=gi, s_=s_: gin_copy(gi, s_, None))
                else:
                    # spread 36 matmuls over the periods of group gi-1,
                    # n-gate blocks first so gin copies can start early
                    p0, pn_ = GROUPS[gi - 1]
                    nper = GROUPS[gi - 1][1]
                    base0 = HEAD + p0 * P
                    at(base0 + XPH - 20,
                       lambda gi=gi: (mk_group_tiles(gi, 0),
                                      mk_group_tiles(gi, 1)))
                    jorder = [(0, 4), (0, 5), (1, 4), (1, 5),
                              (0, 0), (0, 1), (0, 2), (0, 3),
                              (1, 0), (1, 1), (1, 2), (1, 3)]
                    for idx, (s_, j) in enumerate(jorder):
                        per = min(idx // 3, nper - 1)
                        slot = idx % 3
                        tt = base0 + per * P + XPH + slot * 200
                        at(tt, lambda gi=gi, s_=s_, j=j: xmm(gi, s_, j, None))
                        for k in range(2):
                            at(tt + 55 + 55 * k,
                               lambda gi=gi, s_=s_, j=j, k=k:
                               xmm(gi, s_, j, k))
                    # chain-0 gin halves on Act (phase 1310 of periods 1,2);
                    # chain-1 halves on DVE (phase 120 of periods 2,3)
                    mkt = base0 + XPH - 20
                    for hf in range(2):
                        at(max(base0 + min(1 + hf, nper - 1) * P + 1310,
                               mkt + 1500 + hf),
                           lambda gi=gi, hf=hf: gin_copy(gi, 0, hf))
                        at(max(base0 + min(2 + hf, nper - 1) * P + 120,
                               mkt + 1500 + 2 + hf),
                           lambda gi=gi, hf=hf: gin_copy(gi, 1, hf))

            # ---------------- per-step chain ----------------
            def hmms(t, s_, which):
                gi = t // GS
                tl = t % GS
                whT = st['whT']
                hqv = st.get(f'hq{s_}')
                anv = st.get(f'an{s_}')
                if which == 'r':
                    pri = st[f'pri{gi % 2}_{s_}']
                    for j in range(4):
                        for hx in (hqv, anv):
                            for k in range(2):
                                nc.tensor.matmul(
                                    pri[:, j, tl, :], whT[:, k, ts(j, 128)],
                                    hx[:, k, :], start=False, stop=False,
                                    skip_group_check=True)
                elif which == 'n':
                    ps = st[f'bkghn{s_}']
                    st[f'ghn{s_}'] = ps
                    for j in range(2):
                        nc.tensor.matmul(
                            ps[:, j, :], st['bias'][0:1, ts(6 + j, 128)],
                            st['ones'][0:1, ds(0, SW)], start=True,
                            stop=(t == 0))
                        if t > 0:
                            for hi, hx in enumerate((hqv, anv)):
                                for k in range(2):
                                    nc.tensor.matmul(
                                        ps[:, j, :],
                                        whT[:, k, ds(512 + j * 128, 128)],
                                        hx[:, k, :], start=False,
                                        stop=(hi == 1 and k == 1))

            def op_sr(t, s_):
                gi, tl = t // GS, t % GS
                ri_sb = ew.tile([128, 4, SW], BF16, tag=f"ri{s_}",
                                name=f"ri_sb{s_}")
                nc.scalar.activation(ri_sb[:],
                                     st[f'pri{gi % 2}_{s_}'][:, :, tl, :],
                                     AF.Sigmoid)
                st[f'r{s_}'] = ri_sb[:, 0:2, :]
                st[f'i{s_}'] = ri_sb[:, 2:4, :]

            def op_u1(t, s_):
                u1 = ew.tile([128, 2, SW], BF16, tag=f"u1{s_}",
                             name=f"u1_{s_}")
                nc.vector.tensor_tensor(u1[:], st[f'ghn{s_}'][:],
                                        st[f'r{s_}'], OP.mult)
                st[f'u1{s_}'] = u1

            def op_u2(t, s_):
                gi, tl = t // GS, t % GS
                u2 = ew.tile([128, 2, SW], BF16, tag=f"u2{s_}",
                             name=f"u2_{s_}")
                nc.vector.tensor_tensor(u2[:], st[f'u1{s_}'][:],
                                        st[f'gin_g{gi}_{s_}'][:, :, tl, :],
                                        OP.add)
                st[f'u2{s_}'] = u2

            def op_tanh(t, s_):
                n_t = ew.tile([128, 2, SW], BF16, tag=f"n{s_}",
                              name=f"n_t{s_}")
                nc.scalar.activation(n_t[:], st[f'u2{s_}'][:], AF.Tanh)
                st[f'n{s_}'] = n_t

            def op_a(t, s_):
                a_t = ew.tile([128, 2, SW], BF16, tag=f"a{s_}",
                              name=f"a_t{s_}")
                cfsrc = (st['cf0'] if t < 5 else st['cf'])
                cfoff = t * BH + s_ * SW
                nc.gpsimd.tensor_tensor(
                    a_t[:], st[f'i{s_}'],
                    cfsrc[:, None, ds(cfoff, SW)]
                    .to_broadcast((128, 2, SW)), OP.mult)
                st[f'a{s_}'] = a_t

            def op_q(t, s_):
                q_t = ew.tile([128, 2, SW], BF16, tag=f"q{s_}",
                              name=f"q_t{s_}")
                nc.gpsimd.tensor_scalar(q_t[:], st[f'a{s_}'][:], -1.0, 1.0,
                                        OP.mult, OP.add)
                st[f'q{s_}'] = q_t

            def op_hq(t, s_):
                hq = hpool.tile([128, 2, SW], BF16, tag=f"hq{s_}",
                                name=f"hq{s_}")
                nc.vector.tensor_tensor(hq[:], st[f'h{s_}'][:],
                                        st[f'q{s_}'][:], OP.mult)
                st[f'hq{s_}'] = hq

            def op_an(t, s_):
                an = hpool.tile([128, 2, SW], BF16, tag=f"an{s_}",
                                name=f"an{s_}")
                nc.vector.tensor_tensor(an[:], st[f'a{s_}'][:],
                                        st[f'n{s_}'][:], OP.mult)
                st[f'an{s_}'] = an

            def op_hnew(t, s_):
                h_new = hpool.tile([128, 2, SW], BF16, tag=f"h{s_}",
                                   name=f"h_new{s_}")
                eng = nc.vector if t == S - 1 else nc.gpsimd
                eng.tensor_tensor(h_new[:], st[f'hq{s_}'][:],
                                  st[f'an{s_}'][:], OP.add)
                st[f'h{s_}'] = h_new

            for t in range(S):
                for s_ in range(2):
                    ph = PH[s_]
                    base = HEAD + t * P + (OFF_B if s_ else 0)
                    if t > 0:
                        at(base + ph['rmm'], lambda t=t, s_=s_: hmms(t, s_, 'r'),
                           f"rmm.{s_}.{t}")
                    at(base + ph['nmm'], lambda t=t, s_=s_: hmms(t, s_, 'n'),
                       f"nmm.{s_}.{t}")
                    at(base + ph['sr'], lambda t=t, s_=s_: op_sr(t, s_),
                       f"sr.{s_}.{t}")
                    at(base + ph['u1'], lambda t=t, s_=s_: op_u1(t, s_),
                       f"u1.{s_}.{t}")
                    at(base + ph['u2'], lambda t=t, s_=s_: op_u2(t, s_),
                       f"u2.{s_}.{t}")
                    at(base + ph['a'], lambda t=t, s_=s_: op_a(t, s_),
                       f"a.{s_}.{t}")
                    at(base + ph['q'], lambda t=t, s_=s_: op_q(t, s_),
                       f"q.{s_}.{t}")
                    at(base + ph['tanh'], lambda t=t, s_=s_: op_tanh(t, s_),
                       f"tanh.{s_}.{t}")
                    at(base + ph['hq'], lambda t=t, s_=s_: op_hq(t, s_),
                       f"hq.{s_}.{t}")
                    at(base + ph['an'], lambda t=t, s_=s_: op_an(t, s_),
                       f"an.{s_}.{t}")
                    at(base + ph['hnew'], lambda t=t, s_=s_: op_hnew(t, s_),
                       f"hnew.{s_}.{t}")

            def final_dma(s_):
                eng = nc.sync if s_ == 0 else nc.scalar
                eng.dma_start(hn_parts.ap()[:, 0, :, ds(s_ * SW, SW)],
                              st[f'h{s_}'][:])
            for s_ in range(2):
                base49 = HEAD + (S - 1) * P + (OFF_B if s_ else 0)
                at(base49 + PH[s_]['hnew'] + 60,
                   lambda s_=s_: final_dma(s_))

            # ---- emit in planned order ----
            events.sort(key=lambda e: (e[0], e[1]))
            for tns, _, fn, label in events:
                n0 = len(nc.inst_map)
                with tc.tile_wait_until(max(tns, 0) / 1e6):
                    fn()
                if label is not None:
                    for iname in list(nc.inst_map)[n0:]:
                        EMIT_LOG[iname] = (label, tns)

    nc.compile()
    return nc


def scan_host_inputs(seq, emb, emb_purpose, w_ih, w_hh, b_ih, b_hh):
    seq = np.asarray(seq)
    xg = emb[seq]                      # [S, B, D] gather (input staging)
    cs = np.einsum("sbd,pd->sbp", xg, emb_purpose) / TAU
    cs -= cs.max(axis=2, keepdims=True)
    ce = np.exp(cs)
    cw_full = ce / ce.sum(axis=2, keepdims=True)     # [S, B, 3]
    cw_full *= (seq != 0)[:, :, None]
    cw_full *= (cw_full >= EPS)
    HB = 5 * BH
    in_maps = []
    for c in range(NCORES):
        p, hh = CORE_PH[c]
        sl = slice(hh * BH, (hh + 1) * BH)
        xh = xg[:, sl, :]              # [S, BH, D]
        xT = np.ascontiguousarray(
            xh.transpose(2, 0, 1).reshape(2, 128, SB).transpose(1, 0, 2))
        wiT = np.ascontiguousarray(
            w_ih[p].T.reshape(2, 128, 768).transpose(1, 0, 2))
        whT = np.ascontiguousarray(
            w_hh[p].T.reshape(2, 128, 768).transpose(1, 0, 2))
        cf = cw_full[:, sl, p].reshape(SB)
        bias = np.concatenate([
            (b_ih[p] + b_hh[p])[:512],      # r, i combined biases
            b_ih[p][512:],                   # n input bias
            b_hh[p][512:],                   # n hidden bias
        ])
        # blob: [wiT r-rows (2,256) | x cols 0:HB (2,HB) | cf 0:HB bcast]
        blob = np.concatenate([
            wiT[:, :, 0:256].reshape(128, 512),
            xT[:, :, 0:HB].reshape(128, 2 * HB),
            np.broadcast_to(cf[None, 0:HB], (128, HB)),
        ], axis=1)
        in_maps.append({
            "blob": blob.astype(_BF),
            "wiT_in": np.ascontiguousarray(wiT[:, :, 256:768]).astype(_BF),
            "whT": whT.astype(_BF),
            "xT": xT.astype(_BF),
            "cf_lin": cf.reshape(SB // 128, 128).astype(_BF),
            "bias": bias[None, :].astype(_BF),
        })
    return in_maps


# --------------------------------------------------------------------------
# Launch 2: scoring (baseline structure; DMA slice order fixed so each
# q-range's three purposes arrive together)
# --------------------------------------------------------------------------

def build_score_nc():
    nc = bacc.Bacc("TRN2", target_bir_lowering=False, debug=False,
                   num_devices=NCORES)

    hT6_d = nc.dram_tensor("hT6", [128, 6, 128], BF16, kind="ExternalInput")
    scores_d = nc.dram_tensor("scores", [128, T_PAD], BF16,
                              kind="ExternalOutput")

    with tile.TileContext(nc) as tc:
        with (
            tc.tile_pool(name="wetabp", bufs=1, side="right") as wetabp,
            tc.tile_pool(name="consts", bufs=1) as consts,
            tc.tile_pool(name="epsum", bufs=2, space="PSUM") as epsum,
            tc.tile_pool(name="outp", bufs=1) as outp,
        ):
            # same right-side allocation as the scan launch: the tables are
            # already resident in SBUF from launch 1. The pad-column memset
            # marks the tile initialized for dep tracking without touching
            # table data (cols >= 6251 are padding).
            wetab = wetabp.tile([128, 3, 2, T_PAD], BF16, name='wetab')
            nc.vector.memset(wetab[:, 0, 0, ds(T_PAD - 1, 1)], 0.0)
            hT6 = consts.tile([128, 6, 128], BF16)
            nc.sync.dma_start(hT6[:], hT6_d.ap())

            out_sb = outp.tile([128, T_PAD], BF16)
            NCH = len(CHUNK_SIZES)
            for ci, (c0, cs) in enumerate(zip(CHUNK_OFFS[:-1], CHUNK_SIZES)):
                ps = epsum.tile([128, N_CHUNK], F32, tag=f"S{ci % 2}",
                                name="ps_s")
                for p in range(3):
                    for k in range(2):
                        nc.tensor.matmul(
                            ps[:, :cs], hT6[:, p * 2 + k, :],
                            wetab[:, p, k, ds(c0, cs)],
                            start=(p == 0 and k == 0),
                            stop=(p == 2 and k == 1))
                if ci >= NCH - 2:
                    # tail chunks: copy on DVE (off the busy Act queue) and
                    # write back each chunk on its own queue immediately
                    nc.vector.tensor_copy(out_sb[:, ds(c0, cs)], ps[:, :cs])
                    nc.sync.dma_start(scores_d.ap()[:, ds(c0, cs)],
                                      out_sb[:, ds(c0, cs)])
                else:
                    nc.scalar.copy(out_sb[:, ds(c0, cs)], ps[:, :cs])
            WB = [(0, 2), (2, 3), (5, 3), (8, 3)]
            for b0, bn in WB:
                o0 = CHUNK_OFFS[b0]
                o1 = CHUNK_OFFS[b0 + bn]
                nc.scalar.dma_start(scores_d.ap()[:, ds(o0, o1 - o0)],
                                    out_sb[:, ds(o0, o1 - o0)])

    nc.compile()
    return nc


def score_table_inputs(emb, emb_purpose):
    lg = emb @ emb_purpose.T                   # [T, 3]
    e = np.exp(lg - lg.max(axis=1, keepdims=True))
    tcw = (e / e.sum(axis=1, keepdims=True)).astype(np.float32)  # [T, 3]

    base = N_ITEMS // NCORES
    rem = N_ITEMS - base * NCORES
    bounds = []
    s0 = 0
    for c in range(NCORES):
        n = base + (1 if c < rem else 0)
        bounds.append((s0, s0 + n))
        s0 += n

    in_maps = []
    for c in range(NCORES):
        lo, hi = bounds[c]
        n = hi - lo
        m = {}
        for p in range(3):
            we = (emb[lo:hi] * tcw[lo:hi, p:p + 1]).T.astype(_BF)  # [256, n]
            weT = np.zeros((128, 2, T_PAD), _BF)
            weT[:, :, :n] = we.reshape(2, 128, n).transpose(1, 0, 2)
            m[f"weT{p}"] = weT
        in_maps.append(m)
    return in_maps, bounds


# --------------------------------------------------------------------------
# Entry point
# --------------------------------------------------------------------------

_SCAN_NC = None
_SCORE_NC = None


def _wetab_addr(nc):
    for alloc in nc.m.functions[0].allocations:
        if "wetab" in str(getattr(alloc, "name", "")):
            return alloc.memorylocations[0].addr
    raise RuntimeError("wetab allocation not found")


def _get_ncs():
    global _SCAN_NC, _SCORE_NC
    if _SCAN_NC is None:
        _SCAN_NC = build_scan_nc()
    if _SCORE_NC is None:
        _SCORE_NC = build_score_nc()
    assert _wetab_addr(_SCAN_NC) == _wetab_addr(_SCORE_NC), (
        "persistent table region addresses diverged between launches")
    return _SCAN_NC, _SCORE_NC


def kernel(seq, emb, emb_purpose, w_ih, w_hh, b_ih, b_hh):
    seq = np.asarray(seq)
    emb = np.asarray(emb, np.float32)
    emb_purpose = np.asarray(emb_purpose, np.float32)
    w_ih = np.asarray(w_ih, np.float32)
    w_hh = np.asarray(w_hh, np.float32)
    b_ih = np.asarray(b_ih, np.float32)
    b_hh = np.asarray(b_hh, np.float32)

    scan_nc, score_nc = _get_ncs()

    scan_ins = scan_host_inputs(seq, emb, emb_purpose, w_ih, w_hh, b_ih, b_hh)
    tab_ins, bounds = score_table_inputs(emb, emb_purpose)
    for m, t in zip(scan_ins, tab_ins):
        m.update(t)
    res1 = run_bass_kernel_spmd(scan_nc, scan_ins, core_ids=list(range(NCORES)))

    hT6 = np.zeros((128, 6, 128), _BF)
    for c in range(6):
        p, hh = CORE_PH[c]
        sl = res1.results[c]["hn_parts"][:, 0]   # [128, 2, BH]
        for k in range(2):
            hT6[:, p * 2 + k, hh * BH:(hh + 1) * BH] = sl[:, k, :]

    score_ins = [{"hT6": hT6}] * NCORES
    res2 = run_bass_kernel_spmd(score_nc, score_ins, core_ids=list(range(NCORES)))

    scores = np.empty((B, N_ITEMS), np.float32)
    for c in range(NCORES):
        lo, hi = bounds[c]
        scores[:, lo:hi] = res2.results[c]["scores"][:, : hi - lo]\
            .astype(np.float32)
    return scores
